# revision 12
# baseline (speedup 1.0000x reference)
"""Trainium2 Bass SPMD kernel: multi-head attention (B=2, S=2048, D=1024, H=16).

Sharding: data-parallel over batch (2 groups of 4 cores) x tensor-parallel over
heads (4 heads/core). Per core:
  - all matmuls run in fp16 with fp32 PSUM accumulation; fp32-class precision
    on the q/k/scores path comes from hi/lo splitting (x = x_hi + x_lo with
    both halves fp16; products of fp16 pairs are exact in fp32, so
    q_hi*k_hi + q_hi*k_lo + q_lo*k_hi reproduces the fp32 score to ~2^-22).
  - qT/kT computed transposed (head-dim on partitions) from host-transposed
    xT; 1/sqrt(hd) folded into wq host-side (exact power of 2).
  - scores carry an augmented contraction row with the attention-mask bias,
    so masking is free inside the matmul.
  - softmax: DVE row-max (negated) -> ACT exp (bias=-max) with fused row-sum
    accumulation -> DVE reciprocal + scale.  Weights stored fp16.
  - attn @ v: weights transposed on the PE (exact for fp16 values), PV
    accumulates fp32 in PSUM.
  - output projection: per-batch AllGather of attn-outputs (transposed
    layout), then out = merged @ wo in fp16 with fp32 accumulation.
"""
import sys
sys.path.insert(0, '/opt/trn_rl_repo')
import numpy as np
import concourse.bass as bass
import concourse.bacc as bacc
import concourse.tile as tile
from concourse import mybir, masks
from concourse.bass_utils import run_bass_kernel_spmd

F32 = mybir.dt.float32
F16 = mybir.dt.float16
AX = mybir.AxisListType.X
EXP = mybir.ActivationFunctionType.Exp

MASK_NEG = 30000.0  # additive mask bias magnitude (fp16-safe, dwarfs scores)


def build_attention_program(S, HD, HPC, GS, NG):
    """One SPMD program, runs on GS*NG cores.

    S: sequence length; HD: head dim; HPC: heads per core;
    GS: group size (cores per batch, tensor-parallel over heads);
    NG: number of groups (batches).  D = HD * HPC * GS.
    """
    D = HD * HPC * GS
    assert S % 128 == 0 and D % 128 == 0 and HPC % 2 == 0 and HD == 64
    NI = S // 128              # 128-row i-tiles
    JC = min(512, S)           # scores j-chunk (psum bank)
    NJC = S // JC
    IC = min(512, S)           # PV i-chunk
    IPC = IC // 128            # i-tiles per i-chunk
    NK = D // 128              # contraction tiles over model dim
    NJT = S // 128             # j-tiles (transpose blocks / PV accumulation)
    TQ = min(4, NJT)           # transposes per psum bank
    NC_ = min(512, D)          # outproj n-chunk
    NNC = D // NC_
    QC = min(512, S)           # qkv rhs chunk
    NQC = S // QC
    JPQ = QC // 128            # j-tiles per qkv chunk
    NPAIR = HPC // 2
    NSTACK = (HPC * HD) // 128 if HPC * HD >= 128 else 1
    STACK_P = min(128, HPC * HD)
    HEADS_PER_STACK = max(1, 128 // HD) if HPC >= 2 else 1

    nc = bacc.Bacc()
    xth = nc.declare_dram_parameter("xth", [D, S], F16, isOutput=False)
    xtl = nc.declare_dram_parameter("xtl", [D, S], F16, isOutput=False)
    wqh = nc.declare_dram_parameter("wqh", [D, HPC * HD], F16, isOutput=False)
    wql = nc.declare_dram_parameter("wql", [D, HPC * HD], F16, isOutput=False)
    wkh = nc.declare_dram_parameter("wkh", [D, HPC * HD], F16, isOutput=False)
    wkl = nc.declare_dram_parameter("wkl", [D, HPC * HD], F16, isOutput=False)
    wvh = nc.declare_dram_parameter("wvh", [D, HPC * HD], F16, isOutput=False)
    wo = nc.declare_dram_parameter("wo", [D, D], F16, isOutput=False)
    mb = nc.declare_dram_parameter("maskbias", [1, S], F16, isOutput=False)
    w_out = nc.declare_dram_parameter("w_out", [HPC, S, S], F16, isOutput=True)
    ao_out = nc.declare_dram_parameter("ao_out", [HPC * HD, S], F32, isOutput=True)
    y_out = nc.declare_dram_parameter("y_out", [S, D], F32, isOutput=True)

    groups = [[g * GS + r for r in range(GS)] for g in range(NG)]

    with tile.TileContext(nc) as tc:
        with (
            tc.tile_pool(name="persist", bufs=1) as persist,
            tc.tile_pool(name="heads", bufs=1) as heads,
            tc.tile_pool(name="dram", bufs=1, space="DRAM") as drampool,
        ):
            ident = persist.tile([128, 128], F16, tag="ident")
            masks.make_identity(nc, ident[:])

            aoT_dram = drampool.tile([HPC * HD, S], F32, tag="aot")
            ag_drams = []
            for st in range((HPC * HD) // 128 if HPC * HD >= 128 else 1):
                t_ag = drampool.tile([GS * 128, S], F32, tag=f"ag{st}",
                                     name=f"ag_dram{st}")
                ag_drams.append(t_ag)

            # per-head transposed q/k in hi/lo fp16 pairs, with aug row 64:
            #   qTh row64 = 1, kTh row64 = maskbias, qTl/kTl row64 = 0
            qTh, qTl, kTh, kTl, vh = [], [], [], [], []
            for h in range(HPC):
                t_qh = heads.tile([65, S], F16, tag=f"qTh{h}", name=f"qTh{h}")
                t_ql = heads.tile([65, S], F16, tag=f"qTl{h}", name=f"qTl{h}")
                t_kh = heads.tile([65, S], F16, tag=f"kTh{h}", name=f"kTh{h}")
                t_kl = heads.tile([65, S], F16, tag=f"kTl{h}", name=f"kTl{h}")
                t_v = heads.tile([128, NJT * HD], F16, tag=f"v{h}", name=f"v{h}")
                qTh.append(t_qh); qTl.append(t_ql)
                kTh.append(t_kh); kTl.append(t_kl); vh.append(t_v)
                nc.gpsimd.memset(t_qh[64:65, :], 1.0)
                nc.gpsimd.memset(t_ql[64:65, :], 0.0)
                nc.gpsimd.memset(t_kl[64:65, :], 0.0)
                nc.sync.dma_start(t_kh[64:65, :], mb[:])
            aostack = []
            for st in range(NSTACK):
                t_s = persist.tile([STACK_P, S], F32, tag=f"aos{st}", name=f"aos{st}")
                aostack.append(t_s)

            # ---------------- phase 1: q/k/v projections ----------------
            with (
                tc.tile_pool(name="xtp", bufs=2) as xpool,
                tc.tile_pool(name="wp", bufs=1) as wpool,
                tc.tile_pool(name="qkv_ps", bufs=1, space="PSUM") as qkv_ps,
            ):
                wqht, wqlt, wkht, wklt, wvht = [], [], [], [], []
                for kt_i in range(NK):
                    sl = slice(kt_i * 128, (kt_i + 1) * 128)
                    for lst, src, nm in ((wqht, wqh, "wqh"), (wqlt, wql, "wql"),
                                         (wkht, wkh, "wkh"), (wklt, wkl, "wkl"),
                                         (wvht, wvh, "wvh")):
                        t_w = wpool.tile([128, HPC * HD], F16,
                                         tag=f"{nm}{kt_i}", name=f"{nm}{kt_i}")
                        nc.sync.dma_start(t_w[:], src[sl, :])
                        lst.append(t_w)

                for c in range(NQC):
                    csl = slice(c * QC, (c + 1) * QC)
                    xhs, xls = [], []
                    for kt_i in range(NK):
                        sl = slice(kt_i * 128, (kt_i + 1) * 128)
                        t_xh = xpool.tile([128, QC], F16, tag=f"xh{kt_i}",
                                          name=f"xh{kt_i}")
                        nc.sync.dma_start(t_xh[:], xth[sl, csl])
                        t_xl = xpool.tile([128, QC], F16, tag=f"xl{kt_i}",
                                          name=f"xl{kt_i}")
                        nc.sync.dma_start(t_xl[:], xtl[sl, csl])
                        xhs.append(t_xh); xls.append(t_xl)
                    for wht, wlt, dsth, dstl in ((wqht, wqlt, qTh, qTl),
                                                 (wkht, wklt, kTh, kTl)):
                        for p in range(NPAIR):
                            psl = slice(p * 128, (p + 1) * 128)
                            ps_qk = qkv_ps.tile([128, QC], F32, tag="qkps", bufs=3)
                            n_mm = 3 * NK
                            mi = 0
                            for kt_i in range(NK):
                                for wt_, xt_ in ((wht[kt_i], xhs[kt_i]),
                                                 (wht[kt_i], xls[kt_i]),
                                                 (wlt[kt_i], xhs[kt_i])):
                                    nc.tensor.matmul(
                                        ps_qk[:], wt_[:, psl], xt_[:],
                                        start=(mi == 0), stop=(mi == n_mm - 1))
                                    mi += 1
                            # split psum fp32 -> hi fp16 + lo fp16
                            for half, dh, dl in ((slice(0, 64), dsth[2 * p], dstl[2 * p]),
                                                 (slice(64, 128), dsth[2 * p + 1], dstl[2 * p + 1])):
                                nc.scalar.copy(dh[0:64, csl], ps_qk[half, :])
                                nc.vector.tensor_tensor(
                                    out=dl[0:64, csl], in0=ps_qk[half, :],
                                    in1=dh[0:64, csl], op=mybir.AluOpType.subtract)
                    for jj in range(JPQ):
                        jt = c * JPQ + jj
                        jsl = slice(jj * 128, (jj + 1) * 128)
                        ps_v = qkv_ps.tile([128, HPC * HD], F32, tag="vps", bufs=2)
                        for mi, xt_ in enumerate((xhs, xls)):
                            for kt_i in range(NK):
                                nc.tensor.matmul(
                                    ps_v[:], xt_[kt_i][:, jsl], wvht[kt_i][:],
                                    start=(mi == 0 and kt_i == 0),
                                    stop=(mi == 1 and kt_i == NK - 1))
                        for h in range(HPC):
                            nc.any.tensor_copy(
                                vh[h][:, jt * HD:(jt + 1) * HD],
                                ps_v[:, h * HD:(h + 1) * HD])

            # ---------------- phase 2: attention per head ----------------
            with (
                tc.tile_pool(name="wtp", bufs=4) as wtile_pool,
                tc.tile_pool(name="asmp", bufs=2) as asm_pool,
                tc.tile_pool(name="stats", bufs=8) as stats,
                tc.tile_pool(name="sc_ps", bufs=1, space="PSUM") as sc_ps,
                tc.tile_pool(name="tr_ps", bufs=1, space="PSUM") as tr_ps,
                tc.tile_pool(name="pv_ps", bufs=1, space="PSUM") as pv_ps,
            ):
                for h in range(HPC):
                    asm = None
                    for it in range(NI):
                        ic, ipos = divmod(it, IPC)
                        isl = slice(it * 128, (it + 1) * 128)
                        # scores with aug row 64 (mask bias) via hi/lo splits
                        subt = []
                        for jc in range(NJC):
                            jsl = slice(jc * JC, (jc + 1) * JC)
                            ps_sc = sc_ps.tile([128, JC], F32, tag="sc", bufs=5,
                                               name="ps_sc")
                            for mi, (lt, rt) in enumerate(
                                    ((qTh[h], kTh[h]), (qTh[h], kTl[h]),
                                     (qTl[h], kTh[h]))):
                                nc.tensor.matmul(
                                    ps_sc[:], lt[0:65, isl], rt[0:65, jsl],
                                    start=(mi == 0), stop=(mi == 2))
                            subt.append(ps_sc)
                        mx = stats.tile([128, NJC], F32, tag="mx", name="mx")
                        for jc in range(NJC):
                            nc.vector.reduce_max(mx[:, jc:jc + 1], subt[jc][:], axis=AX)
                        nbias = stats.tile([128, 1], F32, tag="nb", name="nbias")
                        nc.vector.reduce_max(nbias[:], mx[:], axis=AX, negate=True)
                        w_t = wtile_pool.tile([128, S], F16, tag="w", name="w_t")
                        sm = stats.tile([128, NJC], F32, tag="sm", name="sm")
                        for jc in range(NJC):
                            nc.scalar.activation(
                                w_t[:, jc * JC:(jc + 1) * JC], subt[jc][:], EXP,
                                bias=nbias[:], scale=1.0,
                                accum_out=sm[:, jc:jc + 1])
                        ssum = stats.tile([128, 1], F32, tag="ss", name="ssum")
                        nc.vector.reduce_sum(ssum[:], sm[:], axis=AX)
                        rec = stats.tile([128, 1], F32, tag="rc", name="rec")
                        nc.vector.reciprocal(rec[:], ssum[:])
                        nc.gpsimd.tensor_scalar_mul(w_t[:], w_t[:], rec[:])
                        nc.sync.dma_start(w_out[h, isl, :], w_t[:])
                        # transpose w into assembly tile (wT: j on partitions)
                        if ipos == 0:
                            asm = asm_pool.tile([128, NJT * IC], F16, tag="asm",
                                                name="asm")
                        asm3 = asm[:].rearrange("p (j i) -> p j i", j=NJT)
                        for q4 in range(NJT // TQ):
                            ps_tr = tr_ps.tile([128, TQ * 128], F16, tag="tr", bufs=2,
                                               name="ps_tr")
                            for d in range(TQ):
                                jt = q4 * TQ + d
                                nc.tensor.matmul(
                                    ps_tr[:, d * 128:(d + 1) * 128],
                                    w_t[:, jt * 128:(jt + 1) * 128],
                                    ident[:], is_transpose=True,
                                    start=True, stop=True)
                            src3 = ps_tr[:].rearrange("p (d c) -> p d c", d=TQ)
                            dst3 = asm3[:, q4 * TQ:(q4 + 1) * TQ,
                                        ipos * 128:(ipos + 1) * 128]
                            if q4 % 4 == 3:
                                nc.vector.tensor_copy(dst3, src3[:])
                            else:
                                nc.scalar.copy(dst3, src3[:])
                        # PV once the assembly holds a full i-chunk
                        if ipos == IPC - 1:
                            ps_pv = pv_ps.tile([64, IC], F32, tag="pv", bufs=1,
                                               name="ps_pv")
                            for jt in range(NJT):
                                nc.tensor.matmul(
                                    ps_pv[:],
                                    vh[h][:, jt * HD:(jt + 1) * HD],
                                    asm3[:, jt, :],
                                    start=(jt == 0), stop=(jt == NJT - 1))
                            st, po = divmod(h, HEADS_PER_STACK)
                            nc.any.tensor_copy(
                                aostack[st][po * HD:(po + 1) * HD,
                                            ic * IC:(ic + 1) * IC],
                                ps_pv[:])
                    # when this head completes a stack, ship it + AllGather it
                    # immediately so the collective overlaps remaining heads
                    st, po = divmod(h, HEADS_PER_STACK)
                    if po == HEADS_PER_STACK - 1 or h == HPC - 1:
                        ssl = slice(st * STACK_P, (st + 1) * STACK_P)
                        nc.sync.dma_start(ao_out[ssl, :], aostack[st][:])
                        nc.sync.dma_start(aoT_dram[ssl, :], aostack[st][:])
                        nc.gpsimd.collective_compute(
                            "AllGather", mybir.AluOpType.bypass,
                            ins=[aoT_dram[ssl, :]], outs=[ag_drams[st][:]],
                            replica_groups=groups)

            # ---------------- phase 3: output projection -----------------

            with (
                tc.tile_pool(name="agp", bufs=1) as ag_pool,
                tc.tile_pool(name="wop", bufs=1) as wo_pool,
                tc.tile_pool(name="outp", bufs=3) as out_pool,
                tc.tile_pool(name="op_ps", bufs=3, space="PSUM") as op_ps,
            ):
                NKO = (GS * HPC * HD) // 128
                # global k-tile order: stack-major so stack-0 tiles (whose
                # AllGather finished first) come first in each accumulation
                kt_order = [(st, r) for st in range(NSTACK) for r in range(GS)]
                agk16, wot = [], []
                for ko, (st, r) in enumerate(kt_order):
                    t_ag = ag_pool.tile([128, S], F32, tag=f"ag{ko}", name=f"ag{ko}")
                    nc.sync.dma_start(
                        t_ag[:], ag_drams[st][r * 128:(r + 1) * 128, :])
                    t_ag16 = ag_pool.tile([128, S], F16, tag=f"ag16_{ko}",
                                          name=f"ag16_{ko}")
                    nc.vector.tensor_copy(t_ag16[:], t_ag[:])
                    agk16.append(t_ag16)
                    # global model-dim rows for (rank r, stack st)
                    row0 = (r * HPC + st * HEADS_PER_STACK) * HD
                    t_wo = wo_pool.tile([128, D], F16, tag=f"wo{ko}", name=f"wo{ko}")
                    nc.sync.dma_start(t_wo[:], wo[row0:row0 + 128, :])
                    wot.append(t_wo)
                for it in range(NI):
                    for ncj in range(NNC):
                        ps_o = op_ps.tile([128, NC_], F32, tag="op", name="ps_o")
                        for kt_i in range(NKO):
                            nc.tensor.matmul(
                                ps_o[:],
                                agk16[kt_i][:, it * 128:(it + 1) * 128],
                                wot[kt_i][:, ncj * NC_:(ncj + 1) * NC_],
                                start=(kt_i == 0), stop=(kt_i == NKO - 1))
                        y_t = out_pool.tile([128, NC_], F32, tag="y", name="y_t")
                        nc.any.tensor_copy(y_t[:], ps_o[:])
                        nc.sync.dma_start(
                            y_out[it * 128:(it + 1) * 128,
                                  ncj * NC_:(ncj + 1) * NC_], y_t[:])
    nc.compile()
    return nc


_CACHE = {}


def _get_program():
    if "nc" not in _CACHE:
        _CACHE["nc"] = build_attention_program(S=2048, HD=64, HPC=4, GS=4, NG=2)
    return _CACHE["nc"]


def _split16(a):
    hi = a.astype(np.float16)
    lo = (a.astype(np.float32) - hi.astype(np.float32)).astype(np.float16)
    return hi, lo


def make_in_maps(x, mask, wq, wk, wv, wo, HPC=4, GS=4, NG=2, HD=64):
    x = np.asarray(x, np.float32)
    mask = np.asarray(mask)
    wq = np.asarray(wq, np.float32)
    wk = np.asarray(wk, np.float32)
    wv = np.asarray(wv, np.float32)
    wo = np.asarray(wo, np.float32)
    scale = np.float32(1.0 / np.sqrt(np.float32(HD)))
    in_maps = []
    xth = {}
    for b in range(NG):
        xth[b] = _split16(np.ascontiguousarray(x[b].T))
    for c in range(GS * NG):
        b, r = divmod(c, GS)
        cols = slice(r * HPC * HD, (r + 1) * HPC * HD)
        qh, ql = _split16(np.ascontiguousarray(wq[:, cols]) * scale)
        kh, kl = _split16(np.ascontiguousarray(wk[:, cols]))
        in_maps.append({
            "xth": xth[b][0], "xtl": xth[b][1],
            "wqh": qh, "wql": ql, "wkh": kh, "wkl": kl,
            "wvh": np.ascontiguousarray(wv[:, cols]).astype(np.float16),
            "wo": wo.astype(np.float16),
            "maskbias": ((mask[b].astype(np.float32) - 1.0)
                         * np.float32(MASK_NEG))[None, :].astype(np.float16),
        })
    return in_maps


def assemble_outputs(results, B=2, S=2048, D=1024, H=16, HPC=4, GS=4, HD=64):
    attn_w = np.empty((B, H, S, S), np.float32)
    attn_o = np.empty((B, H, S, HD), np.float32)
    out = np.empty((B, S, D), np.float32)
    for c in range(len(results)):
        b, r = divmod(c, GS)
        hs = slice(r * HPC, (r + 1) * HPC)
        attn_w[b, hs] = results[c]["w_out"].astype(np.float32)
        aoT = results[c]["ao_out"].reshape(HPC, HD, S)
        attn_o[b, hs] = aoT.transpose(0, 2, 1)
        if r == 0:
            out[b] = results[c]["y_out"]
    return out, attn_w, attn_o


def kernel(x, mask, wq, wk, wv, wo):
    nc = _get_program()
    in_maps = make_in_maps(x, mask, wq, wk, wv, wo)
    res = run_bass_kernel_spmd(nc, in_maps, list(range(8)))
    return assemble_outputs(res.results)


if __name__ == "__main__":
    nc = _get_program()
    print("program built ok")


# revision 13
# speedup vs baseline: 18.2609x; 18.2609x over previous
"""Trainium2 Bass SPMD kernel: multi-head attention (B=2, S=2048, D=1024, H=16).

Sharding: data-parallel over batch (2 groups of 4 cores) x tensor-parallel over
heads (4 heads/core). Per core:
  - all matmuls run in fp16 with fp32 PSUM accumulation; fp32-class precision
    on the q/k/scores path comes from hi/lo splitting (x = x_hi + x_lo with
    both halves fp16; products of fp16 pairs are exact in fp32, so
    q_hi*k_hi + q_hi*k_lo + q_lo*k_hi reproduces the fp32 score to ~2^-22).
  - qT/kT computed transposed (head-dim on partitions) from host-transposed
    xT; 1/sqrt(hd) folded into wq host-side (exact power of 2).
  - scores carry an augmented contraction row with the attention-mask bias,
    so masking is free inside the matmul.
  - softmax: DVE row-max (negated) -> ACT exp (bias=-max) with fused row-sum
    accumulation -> DVE reciprocal + scale.  Weights stored fp16.
  - attn @ v: weights transposed on the PE (exact for fp16 values), PV
    accumulates fp32 in PSUM.
  - output projection: per-batch AllGather of attn-outputs (transposed
    layout), then out = merged @ wo in fp16 with fp32 accumulation.
"""
import sys
sys.path.insert(0, '/opt/trn_rl_repo')
import numpy as np
import concourse.bass as bass
import concourse.bacc as bacc
import concourse.tile as tile
from concourse import mybir, masks
from concourse.bass_utils import run_bass_kernel_spmd

F32 = mybir.dt.float32
F16 = mybir.dt.float16
AX = mybir.AxisListType.X
EXP = mybir.ActivationFunctionType.Exp

MASK_NEG = 30000.0  # additive mask bias magnitude (fp16-safe, dwarfs scores)


def build_attention_program(S, HD, HPC, GS, NG):
    """One SPMD program, runs on GS*NG cores.

    S: sequence length; HD: head dim; HPC: heads per core;
    GS: group size (cores per batch, tensor-parallel over heads);
    NG: number of groups (batches).  D = HD * HPC * GS.
    """
    D = HD * HPC * GS
    assert S % 128 == 0 and D % 128 == 0 and HPC % 2 == 0 and HD == 64
    NI = S // 128              # 128-row i-tiles
    JC = min(512, S)           # scores j-chunk (psum bank)
    NJC = S // JC
    IC = min(512, S)           # PV i-chunk
    IPC = IC // 128            # i-tiles per i-chunk
    NK = D // 128              # contraction tiles over model dim
    NJT = S // 128             # j-tiles (transpose blocks / PV accumulation)
    TQ = min(4, NJT)           # transposes per psum bank
    NC_ = min(512, D)          # outproj n-chunk
    NNC = D // NC_
    QC = min(512, S)           # qkv rhs chunk
    NQC = S // QC
    JPQ = QC // 128            # j-tiles per qkv chunk
    NPAIR = HPC // 2
    NSTACK = (HPC * HD) // 128 if HPC * HD >= 128 else 1
    STACK_P = min(128, HPC * HD)
    HEADS_PER_STACK = max(1, 128 // HD) if HPC >= 2 else 1

    nc = bacc.Bacc()
    xth = nc.declare_dram_parameter("xth", [D, S], F16, isOutput=False)
    xtl = nc.declare_dram_parameter("xtl", [D, S], F16, isOutput=False)
    wqh = nc.declare_dram_parameter("wqh", [D, HPC * HD], F16, isOutput=False)
    wql = nc.declare_dram_parameter("wql", [D, HPC * HD], F16, isOutput=False)
    wkh = nc.declare_dram_parameter("wkh", [D, HPC * HD], F16, isOutput=False)
    wkl = nc.declare_dram_parameter("wkl", [D, HPC * HD], F16, isOutput=False)
    wvh = nc.declare_dram_parameter("wvh", [D, HPC * HD], F16, isOutput=False)
    wo = nc.declare_dram_parameter("wo", [D, D], F16, isOutput=False)
    mb = nc.declare_dram_parameter("maskbias", [1, S], F16, isOutput=False)
    w_out = nc.declare_dram_parameter("w_out", [HPC, S, S], F16, isOutput=True)
    ao_out = nc.declare_dram_parameter("ao_out", [HPC * HD, S], F32, isOutput=True)
    y_out = nc.declare_dram_parameter("y_out", [S, D], F32, isOutput=True)

    groups = [[g * GS + r for r in range(GS)] for g in range(NG)]

    with tile.TileContext(nc) as tc:
        with (
            tc.tile_pool(name="persist", bufs=1) as persist,
            tc.tile_pool(name="heads", bufs=1) as heads,
            tc.tile_pool(name="dram", bufs=1, space="DRAM") as drampool,
        ):
            ident = persist.tile([128, 128], F16, tag="ident")
            masks.make_identity(nc, ident[:])

            aoT_dram = drampool.tile([HPC * HD, S], F32, tag="aot")
            ag_drams = []
            for st in range((HPC * HD) // 128 if HPC * HD >= 128 else 1):
                t_ag = drampool.tile([GS * 128, S], F32, tag=f"ag{st}",
                                     name=f"ag_dram{st}")
                ag_drams.append(t_ag)

            # per-head transposed q/k in hi/lo fp16 pairs, with aug row 64:
            #   qTh row64 = 1, kTh row64 = maskbias, qTl/kTl row64 = 0
            qTh, qTl, kTh, kTl, vh = [], [], [], [], []
            for h in range(HPC):
                t_qh = heads.tile([65, S], F16, tag=f"qTh{h}", name=f"qTh{h}")
                t_ql = heads.tile([65, S], F16, tag=f"qTl{h}", name=f"qTl{h}")
                t_kh = heads.tile([65, S], F16, tag=f"kTh{h}", name=f"kTh{h}")
                t_kl = heads.tile([65, S], F16, tag=f"kTl{h}", name=f"kTl{h}")
                t_v = heads.tile([128, NJT * HD], F16, tag=f"v{h}", name=f"v{h}")
                qTh.append(t_qh); qTl.append(t_ql)
                kTh.append(t_kh); kTl.append(t_kl); vh.append(t_v)
                nc.gpsimd.memset(t_qh[64:65, :], 1.0)
                nc.gpsimd.memset(t_ql[64:65, :], 0.0)
                nc.gpsimd.memset(t_kl[64:65, :], 0.0)
                nc.sync.dma_start(t_kh[64:65, :], mb[:])
            aostack = []
            for st in range(NSTACK):
                t_s = persist.tile([STACK_P, S], F32, tag=f"aos{st}", name=f"aos{st}")
                aostack.append(t_s)

            # ---------------- phase 1: q/k/v projections ----------------
            with (
                tc.tile_pool(name="xtp", bufs=2) as xpool,
                tc.tile_pool(name="wp", bufs=1) as wpool,
                tc.tile_pool(name="qkv_ps", bufs=1, space="PSUM") as qkv_ps,
            ):
                wqht, wqlt, wkht, wklt, wvht = [], [], [], [], []
                for kt_i in range(NK):
                    sl = slice(kt_i * 128, (kt_i + 1) * 128)
                    for lst, src, nm in ((wqht, wqh, "wqh"), (wqlt, wql, "wql"),
                                         (wkht, wkh, "wkh"), (wklt, wkl, "wkl"),
                                         (wvht, wvh, "wvh")):
                        t_w = wpool.tile([128, HPC * HD], F16,
                                         tag=f"{nm}{kt_i}", name=f"{nm}{kt_i}")
                        nc.sync.dma_start(t_w[:], src[sl, :])
                        lst.append(t_w)

                for c in range(NQC):
                    csl = slice(c * QC, (c + 1) * QC)
                    xhs, xls = [], []
                    for kt_i in range(NK):
                        sl = slice(kt_i * 128, (kt_i + 1) * 128)
                        t_xh = xpool.tile([128, QC], F16, tag=f"xh{kt_i}",
                                          name=f"xh{kt_i}")
                        nc.sync.dma_start(t_xh[:], xth[sl, csl])
                        t_xl = xpool.tile([128, QC], F16, tag=f"xl{kt_i}",
                                          name=f"xl{kt_i}")
                        nc.sync.dma_start(t_xl[:], xtl[sl, csl])
                        xhs.append(t_xh); xls.append(t_xl)
                    for wht, wlt, dsth, dstl in ((wqht, wqlt, qTh, qTl),
                                                 (wkht, wklt, kTh, kTl)):
                        for p in range(NPAIR):
                            psl = slice(p * 128, (p + 1) * 128)
                            ps_qk = qkv_ps.tile([128, QC], F32, tag="qkps", bufs=3)
                            n_mm = 3 * NK
                            mi = 0
                            for kt_i in range(NK):
                                for wt_, xt_ in ((wht[kt_i], xhs[kt_i]),
                                                 (wht[kt_i], xls[kt_i]),
                                                 (wlt[kt_i], xhs[kt_i])):
                                    nc.tensor.matmul(
                                        ps_qk[:], wt_[:, psl], xt_[:],
                                        start=(mi == 0), stop=(mi == n_mm - 1))
                                    mi += 1
                            # split psum fp32 -> hi fp16 + lo fp16
                            for half, dh, dl in ((slice(0, 64), dsth[2 * p], dstl[2 * p]),
                                                 (slice(64, 128), dsth[2 * p + 1], dstl[2 * p + 1])):
                                nc.scalar.copy(dh[0:64, csl], ps_qk[half, :])
                                nc.vector.tensor_tensor(
                                    out=dl[0:64, csl], in0=ps_qk[half, :],
                                    in1=dh[0:64, csl], op=mybir.AluOpType.subtract)
                    for jj in range(JPQ):
                        jt = c * JPQ + jj
                        jsl = slice(jj * 128, (jj + 1) * 128)
                        ps_v = qkv_ps.tile([128, HPC * HD], F32, tag="vps", bufs=2)
                        for mi, xt_ in enumerate((xhs, xls)):
                            for kt_i in range(NK):
                                nc.tensor.matmul(
                                    ps_v[:], xt_[kt_i][:, jsl], wvht[kt_i][:],
                                    start=(mi == 0 and kt_i == 0),
                                    stop=(mi == 1 and kt_i == NK - 1))
                        for h in range(HPC):
                            nc.any.tensor_copy(
                                vh[h][:, jt * HD:(jt + 1) * HD],
                                ps_v[:, h * HD:(h + 1) * HD])

            # ---------------- phase 2: attention per head ----------------
            with (
                tc.tile_pool(name="wtp", bufs=4) as wtile_pool,
                tc.tile_pool(name="asmp", bufs=2) as asm_pool,
                tc.tile_pool(name="stats", bufs=8) as stats,
                tc.tile_pool(name="sc_ps", bufs=1, space="PSUM") as sc_ps,
                tc.tile_pool(name="tr_ps", bufs=1, space="PSUM") as tr_ps,
                tc.tile_pool(name="pv_ps", bufs=1, space="PSUM") as pv_ps,
            ):
                for h in range(HPC):
                    asm = None
                    for it in range(NI):
                        ic, ipos = divmod(it, IPC)
                        isl = slice(it * 128, (it + 1) * 128)
                        # scores with aug row 64 (mask bias) via hi/lo splits
                        subt = []
                        for jc in range(NJC):
                            jsl = slice(jc * JC, (jc + 1) * JC)
                            ps_sc = sc_ps.tile([128, JC], F32, tag="sc", bufs=5,
                                               name="ps_sc")
                            for mi, (lt, rt) in enumerate(
                                    ((qTh[h], kTh[h]), (qTh[h], kTl[h]),
                                     (qTl[h], kTh[h]))):
                                nc.tensor.matmul(
                                    ps_sc[:], lt[0:65, isl], rt[0:65, jsl],
                                    start=(mi == 0), stop=(mi == 2))
                            subt.append(ps_sc)
                        mx = stats.tile([128, NJC], F32, tag="mx", name="mx")
                        for jc in range(NJC):
                            nc.vector.reduce_max(mx[:, jc:jc + 1], subt[jc][:], axis=AX)
                        nbias = stats.tile([128, 1], F32, tag="nb", name="nbias")
                        nc.vector.reduce_max(nbias[:], mx[:], axis=AX, negate=True)
                        w_t = wtile_pool.tile([128, S], F16, tag="w", name="w_t")
                        sm = stats.tile([128, NJC], F32, tag="sm", name="sm")
                        for jc in range(NJC):
                            nc.scalar.activation(
                                w_t[:, jc * JC:(jc + 1) * JC], subt[jc][:], EXP,
                                bias=nbias[:], scale=1.0,
                                accum_out=sm[:, jc:jc + 1])
                        ssum = stats.tile([128, 1], F32, tag="ss", name="ssum")
                        nc.vector.reduce_sum(ssum[:], sm[:], axis=AX)
                        rec = stats.tile([128, 1], F32, tag="rc", name="rec")
                        nc.vector.reciprocal(rec[:], ssum[:])
                        nc.vector.tensor_scalar_mul(w_t[:], w_t[:], rec[:])
                        nc.sync.dma_start(w_out[h, isl, :], w_t[:])
                        # transpose w into assembly tile (wT: j on partitions)
                        if ipos == 0:
                            asm = asm_pool.tile([128, NJT * IC], F16, tag="asm",
                                                name="asm")
                        asm3 = asm[:].rearrange("p (j i) -> p j i", j=NJT)
                        for q4 in range(NJT // TQ):
                            ps_tr = tr_ps.tile([128, TQ * 128], F16, tag="tr", bufs=2,
                                               name="ps_tr")
                            for d in range(TQ):
                                jt = q4 * TQ + d
                                nc.tensor.matmul(
                                    ps_tr[:, d * 128:(d + 1) * 128],
                                    w_t[:, jt * 128:(jt + 1) * 128],
                                    ident[:], is_transpose=True,
                                    start=True, stop=True)
                            src3 = ps_tr[:].rearrange("p (d c) -> p d c", d=TQ)
                            dst3 = asm3[:, q4 * TQ:(q4 + 1) * TQ,
                                        ipos * 128:(ipos + 1) * 128]
                            if q4 % 4 == 3:
                                nc.vector.tensor_copy(dst3, src3[:])
                            else:
                                nc.scalar.copy(dst3, src3[:])
                        # PV once the assembly holds a full i-chunk
                        if ipos == IPC - 1:
                            ps_pv = pv_ps.tile([64, IC], F32, tag="pv", bufs=1,
                                               name="ps_pv")
                            for jt in range(NJT):
                                nc.tensor.matmul(
                                    ps_pv[:],
                                    vh[h][:, jt * HD:(jt + 1) * HD],
                                    asm3[:, jt, :],
                                    start=(jt == 0), stop=(jt == NJT - 1))
                            st, po = divmod(h, HEADS_PER_STACK)
                            nc.any.tensor_copy(
                                aostack[st][po * HD:(po + 1) * HD,
                                            ic * IC:(ic + 1) * IC],
                                ps_pv[:])
                    # when this head completes a stack, ship it + AllGather it
                    # immediately so the collective overlaps remaining heads
                    st, po = divmod(h, HEADS_PER_STACK)
                    if po == HEADS_PER_STACK - 1 or h == HPC - 1:
                        ssl = slice(st * STACK_P, (st + 1) * STACK_P)
                        nc.sync.dma_start(ao_out[ssl, :], aostack[st][:])
                        nc.sync.dma_start(aoT_dram[ssl, :], aostack[st][:])
                        nc.gpsimd.collective_compute(
                            "AllGather", mybir.AluOpType.bypass,
                            ins=[aoT_dram[ssl, :]], outs=[ag_drams[st][:]],
                            replica_groups=groups)

            # ---------------- phase 3: output projection -----------------

            with (
                tc.tile_pool(name="agp", bufs=1) as ag_pool,
                tc.tile_pool(name="wop", bufs=1) as wo_pool,
                tc.tile_pool(name="outp", bufs=3) as out_pool,
                tc.tile_pool(name="op_ps", bufs=3, space="PSUM") as op_ps,
            ):
                NKO = (GS * HPC * HD) // 128
                # global k-tile order: stack-major so stack-0 tiles (whose
                # AllGather finished first) come first in each accumulation
                kt_order = [(st, r) for st in range(NSTACK) for r in range(GS)]
                agk16, wot = [], []
                for ko, (st, r) in enumerate(kt_order):
                    t_ag = ag_pool.tile([128, S], F32, tag=f"ag{ko}", name=f"ag{ko}")
                    nc.sync.dma_start(
                        t_ag[:], ag_drams[st][r * 128:(r + 1) * 128, :])
                    t_ag16 = ag_pool.tile([128, S], F16, tag=f"ag16_{ko}",
                                          name=f"ag16_{ko}")
                    nc.vector.tensor_copy(t_ag16[:], t_ag[:])
                    agk16.append(t_ag16)
                    # global model-dim rows for (rank r, stack st)
                    row0 = (r * HPC + st * HEADS_PER_STACK) * HD
                    t_wo = wo_pool.tile([128, D], F16, tag=f"wo{ko}", name=f"wo{ko}")
                    nc.sync.dma_start(t_wo[:], wo[row0:row0 + 128, :])
                    wot.append(t_wo)
                for it in range(NI):
                    for ncj in range(NNC):
                        ps_o = op_ps.tile([128, NC_], F32, tag="op", name="ps_o")
                        for kt_i in range(NKO):
                            nc.tensor.matmul(
                                ps_o[:],
                                agk16[kt_i][:, it * 128:(it + 1) * 128],
                                wot[kt_i][:, ncj * NC_:(ncj + 1) * NC_],
                                start=(kt_i == 0), stop=(kt_i == NKO - 1))
                        y_t = out_pool.tile([128, NC_], F32, tag="y", name="y_t")
                        nc.any.tensor_copy(y_t[:], ps_o[:])
                        nc.sync.dma_start(
                            y_out[it * 128:(it + 1) * 128,
                                  ncj * NC_:(ncj + 1) * NC_], y_t[:])
    nc.compile()
    return nc


_CACHE = {}


def _get_program():
    if "nc" not in _CACHE:
        _CACHE["nc"] = build_attention_program(S=2048, HD=64, HPC=4, GS=4, NG=2)
    return _CACHE["nc"]


def _split16(a):
    hi = a.astype(np.float16)
    lo = (a.astype(np.float32) - hi.astype(np.float32)).astype(np.float16)
    return hi, lo


def make_in_maps(x, mask, wq, wk, wv, wo, HPC=4, GS=4, NG=2, HD=64):
    x = np.asarray(x, np.float32)
    mask = np.asarray(mask)
    wq = np.asarray(wq, np.float32)
    wk = np.asarray(wk, np.float32)
    wv = np.asarray(wv, np.float32)
    wo = np.asarray(wo, np.float32)
    scale = np.float32(1.0 / np.sqrt(np.float32(HD)))
    in_maps = []
    xth = {}
    for b in range(NG):
        xth[b] = _split16(np.ascontiguousarray(x[b].T))
    for c in range(GS * NG):
        b, r = divmod(c, GS)
        cols = slice(r * HPC * HD, (r + 1) * HPC * HD)
        qh, ql = _split16(np.ascontiguousarray(wq[:, cols]) * scale)
        kh, kl = _split16(np.ascontiguousarray(wk[:, cols]))
        in_maps.append({
            "xth": xth[b][0], "xtl": xth[b][1],
            "wqh": qh, "wql": ql, "wkh": kh, "wkl": kl,
            "wvh": np.ascontiguousarray(wv[:, cols]).astype(np.float16),
            "wo": wo.astype(np.float16),
            "maskbias": ((mask[b].astype(np.float32) - 1.0)
                         * np.float32(MASK_NEG))[None, :].astype(np.float16),
        })
    return in_maps


def assemble_outputs(results, B=2, S=2048, D=1024, H=16, HPC=4, GS=4, HD=64):
    attn_w = np.empty((B, H, S, S), np.float32)
    attn_o = np.empty((B, H, S, HD), np.float32)
    out = np.empty((B, S, D), np.float32)
    for c in range(len(results)):
        b, r = divmod(c, GS)
        hs = slice(r * HPC, (r + 1) * HPC)
        attn_w[b, hs] = results[c]["w_out"].astype(np.float32)
        aoT = results[c]["ao_out"].reshape(HPC, HD, S)
        attn_o[b, hs] = aoT.transpose(0, 2, 1)
        if r == 0:
            out[b] = results[c]["y_out"]
    return out, attn_w, attn_o


def kernel(x, mask, wq, wk, wv, wo):
    nc = _get_program()
    in_maps = make_in_maps(x, mask, wq, wk, wv, wo)
    res = run_bass_kernel_spmd(nc, in_maps, list(range(8)))
    return assemble_outputs(res.results)


if __name__ == "__main__":
    nc = _get_program()
    print("program built ok")
